# revision 6
# baseline (speedup 1.0000x reference)
"""Trainium2 Bass kernel for the neural-CA model (nn_CAModel_optimizedTraining).

Strategy (per core, data-parallel over batch B=8 across 8 cores):
  - Image [256,256,16] stored in SBUF as 8 horizontal bands (32 rows each,
    plus 1-row halos and 1-col zero pads) packed onto 128 partitions:
    lane(g, c) = 32*(g%4) + 16*(g//4) + c.
  - Sobel convs are separable. DVE computes the two w-direction passes
    S' = x(w-1)+x(w+1) and T = x(w+1)-x(w-1); the h-direction combination
    and the channel-concat matmul are folded into 3 "tap" matmuls
    (dh in {-1,0,+1}) with host-precomputed [48,128] weights over a
    DMA-stacked [x; S'; T] buffer. Row-tiled 2x on the PE (two bands at
    tile_position (0,0) / (64,0)).
  - Hidden relu+bias fused into the PSUM->SBUF pass (ScalarE/VectorE).
  - Second matmul W1@h col-tiled 4x with M=32 zero-padded weights so the
    per-band dx lands densely in one [128,n] PSUM tile (rows 0..2 of W1
    zeroed so fixed channels never change).
  - Stochastic fire mask is input-independent (jax threefry on host),
    uploaded pre-broadcast in the band layout; pad columns get mask 0 so
    padding stays exactly zero.
"""

import os
import sys

import numpy as np

H, W, C, HID = 256, 256, 16, 128
NB = 8            # bands per core
BH = H // NB      # 32 rows per band
WP = W + 2        # padded row width
RPB = BH + 2      # rows per band incl halos
F = RPB * WP      # flat band length (8772)
FI = BH * WP      # interior span (8256)
F0 = WP           # interior start offset
NCORES = 8
FIRE_RATE = 0.5
CHUNK = 512


def _lane(g):
    return 32 * (g % 4) + 16 * (g // 4)


def _chunks():
    out = []
    off = 0
    while off < FI:
        n = min(CHUNK, FI - off)
        out.append((off, n))
        off += n
    return out


# ---------------------------------------------------------------- host prep

def _masks(steps):
    """Reference-exact fire masks: [steps, B, H, W] float32 in {0,1}."""
    import jax
    with jax.default_device(jax.devices("cpu")[0]):
        key = jax.random.key(42)
        ms = []
        for step in range(steps):
            mk = jax.random.fold_in(key, step)
            u = jax.random.uniform(mk, (NCORES, H, W, 1))
            ms.append(np.asarray(u[..., 0] > FIRE_RATE, dtype=np.float32))
    return np.stack(ms) if steps else np.zeros((0, NCORES, H, W), np.float32)


def _build_xp(xb):
    """[H,W,C] -> [128, F] padded band layout."""
    xp = np.zeros((128, F), dtype=np.float32)
    band = np.zeros((NB, RPB, WP, C), dtype=np.float32)
    for g in range(NB):
        band[g, 1:BH + 1, 1:W + 1, :] = xb[32 * g:32 * g + BH]
        if g > 0:
            band[g, 0, 1:W + 1, :] = xb[32 * g - 1]
        if g < NB - 1:
            band[g, BH + 1, 1:W + 1, :] = xb[32 * g + BH]
    flat = band.transpose(0, 3, 1, 2).reshape(NB, C, F)
    for g in range(NB):
        xp[_lane(g):_lane(g) + C, :] = flat[g]
    return xp


def _unbuild_xp(xp):
    """[128, F] -> [H,W,C]."""
    out = np.empty((H, W, C), dtype=np.float32)
    for g in range(NB):
        fl = xp[_lane(g):_lane(g) + C, :].reshape(C, RPB, WP)
        out[32 * g:32 * g + BH] = fl[:, 1:BH + 1, 1:W + 1].transpose(1, 2, 0)
    return out


def _build_mask(mb):
    """[H,W] {0,1} -> [128, FI] band layout (pad cols -> 0)."""
    mx = np.zeros((128, FI), dtype=np.float32)
    for g in range(NB):
        mrows = np.zeros((BH, WP), dtype=np.float32)
        mrows[:, 1:W + 1] = mb[32 * g:32 * g + BH]
        mx[_lane(g):_lane(g) + C, :] = mrows.reshape(1, FI)
    return mx


def _tap_weights(W0):
    """[128, 3*128] bf16 tap weights (rows 0:48 strip A, 64:112 strip B)."""
    import ml_dtypes
    W0a = W0[:, 0:C].astype(np.float64)
    W0b = W0[:, C:2 * C].astype(np.float64)
    W0c = W0[:, 2 * C:3 * C].astype(np.float64)
    z = np.zeros((C, HID), np.float64)
    taps = [
        np.vstack([(-2 / 8) * W0b.T, (-1 / 8) * W0b.T, (1 / 8) * W0c.T]),   # dh=-1
        np.vstack([W0a.T,            z,                (2 / 8) * W0c.T]),   # dh= 0
        np.vstack([(2 / 8) * W0b.T,  (1 / 8) * W0b.T,  (1 / 8) * W0c.T]),   # dh=+1
    ]
    wt = np.zeros((128, 3 * 128), dtype=np.float32)
    for i, tp in enumerate(taps):
        wt[0:48, 128 * i:128 * i + 128] = tp
        wt[64:112, 128 * i:128 * i + 128] = tp
    return wt.astype(ml_dtypes.bfloat16)


def _mm2_weights(W1):
    """[128, 64] bf16: [:,0:32] round-1 [W1z^T | 0], [:,32:64] round-2."""
    import ml_dtypes
    W1z = W1.astype(np.float32).copy()
    W1z[0:3, :] = 0.0
    w2 = np.zeros((128, 64), dtype=np.float32)
    w2[:, 0:16] = W1z.T
    w2[:, 48:64] = W1z.T
    return w2.astype(ml_dtypes.bfloat16)


# ---------------------------------------------------------------- bass build

def _build_program(steps, h_on_dve_mod=3):
    """Returns (nc, names). h_on_dve_mod: every k-th h-pass op goes to DVE."""
    from contextlib import ExitStack
    import concourse.bass as bass
    import concourse.tile as tile
    from concourse import bacc, mybir

    nc = bacc.Bacc("TRN2", target_bir_lowering=False)
    dt = mybir.dt
    xp_in = nc.dram_tensor("xp", [128, F], dt.float32, kind="ExternalInput")
    mask_in = nc.dram_tensor("mask", [max(steps, 1), 128, FI], dt.float32,
                             kind="ExternalInput")
    wt_in = nc.dram_tensor("wt", [128, 3 * 128], dt.bfloat16, kind="ExternalInput")
    w2_in = nc.dram_tensor("w2", [128, 64], dt.bfloat16, kind="ExternalInput")
    b0_in = nc.dram_tensor("b0", [128, 1], dt.float32, kind="ExternalInput")
    out_d = nc.dram_tensor("out", [128, F], dt.float32, kind="ExternalOutput")

    with tile.TileContext(nc) as tc, ExitStack() as ctx:
        const = ctx.enter_context(tc.tile_pool(name="const", bufs=1))
        state = ctx.enter_context(tc.tile_pool(name="state", bufs=1))
        hpool = ctx.enter_context(tc.tile_pool(name="hpool", bufs=16))
        mpool = ctx.enter_context(tc.tile_pool(name="mpool", bufs=4))
        dpool = ctx.enter_context(tc.tile_pool(name="dpool", bufs=4))
        pshp = ctx.enter_context(tc.tile_pool(name="pshp", bufs=4, space="PSUM"))
        psdx = ctx.enter_context(tc.tile_pool(name="psdx", bufs=2, space="PSUM"))

        wtt = const.tile([128, 3 * 128], dt.bfloat16)
        w2t = const.tile([128, 64], dt.bfloat16)
        b0t = const.tile([128, 1], dt.float32)
        nc.sync.dma_start(wtt[:, :], wt_in.ap())
        nc.sync.dma_start(w2t[:, :], w2_in.ap())
        nc.sync.dma_start(b0t[:, :], b0_in.ap())

        XP = state.tile([128, F], dt.float32)
        S = state.tile([128, F], dt.bfloat16)
        T = state.tile([128, F], dt.bfloat16)
        YSTs = [state.tile([128, F], dt.bfloat16, name=f"yst{p}") for p in range(4)]
        nc.sync.dma_start(XP[:, :], xp_in.ap())
        for t_ in (S, T):
            nc.vector.memset(t_[:, 0:1], 0.0)
            nc.vector.memset(t_[:, F - 1:F], 0.0)

        Relu = mybir.ActivationFunctionType.Relu
        add_op, max_op = mybir.AluOpType.add, mybir.AluOpType.max

        hctr = 0
        for s in range(steps):
            # w-direction conv passes
            nc.vector.tensor_add(S[:, 1:F - 1], XP[:, 0:F - 2], XP[:, 2:F])
            nc.vector.tensor_sub(T[:, 1:F - 1], XP[:, 2:F], XP[:, 0:F - 2])

            # stack [x; S'; T] per band-pair (strip A rows 0:48, B rows 64:112)
            for p in range(4):
                gA, gB = 2 * p, 2 * p + 1
                for base, g in ((0, gA), (64, gB)):
                    ln = _lane(g)
                    nc.gpsimd.dma_start(YSTs[p][base:base + 16, :],
                                        XP[ln:ln + 16, :])       # cast f32->bf16
                    nc.sync.dma_start(YSTs[p][base + 16:base + 32, :],
                                      S[ln:ln + 16, :])
                    nc.sync.dma_start(YSTs[p][base + 32:base + 48, :],
                                      T[ln:ln + 16, :])

            for (off, n) in _chunks():
                mt = mpool.tile([128, CHUNK], dt.float32, tag="mt")
                nc.sync.dma_start(mt[:, 0:n], mask_in.ap()[s, :, off:off + n])

                hs = []
                for p in range(4):
                    pa = pshp.tile([128, CHUNK], dt.float32, tag="ph")
                    pb = pshp.tile([128, CHUNK], dt.float32, tag="ph")
                    for i in range(3):
                        woff = off + 258 * i
                        nc.tensor.matmul(pa[:, 0:n],
                                         wtt[0:48, 128 * i:128 * i + 128],
                                         YSTs[p][0:48, woff:woff + n],
                                         start=(i == 0), stop=(i == 2),
                                         tile_position=(0, 0))
                        nc.tensor.matmul(pb[:, 0:n],
                                         wtt[64:112, 128 * i:128 * i + 128],
                                         YSTs[p][64:112, woff:woff + n],
                                         start=(i == 0), stop=(i == 2),
                                         tile_position=(64, 0))
                    for ps in (pa, pb):
                        ht = hpool.tile([128, CHUNK], dt.bfloat16, tag="h")
                        if h_on_dve_mod and hctr % h_on_dve_mod == 0:
                            nc.vector.tensor_scalar(ht[:, 0:n], ps[:, 0:n],
                                                    b0t[:, 0:1], 0.0, add_op, max_op)
                        else:
                            nc.scalar.activation(ht[:, 0:n], ps[:, 0:n], Relu,
                                                 bias=b0t[:, 0:1])
                        hctr += 1
                        hs.append(ht)

                pdx = psdx.tile([128, CHUNK], dt.float32, tag="pdx")
                for q in range(4):
                    nc.tensor.matmul(pdx[32 * q:32 * q + 32, 0:n], w2t[:, 0:32],
                                     hs[q][:, 0:n], start=True, stop=False,
                                     tile_position=(0, 32 * q),
                                     skip_group_check=True)
                for q in range(4):
                    nc.tensor.matmul(pdx[32 * q:32 * q + 32, 0:n], w2t[:, 32:64],
                                     hs[4 + q][:, 0:n], start=False, stop=True,
                                     tile_position=(0, 32 * q),
                                     skip_group_check=True)

                dxm = dpool.tile([128, CHUNK], dt.float32, tag="dxm")
                nc.vector.tensor_mul(dxm[:, 0:n], pdx[:, 0:n], mt[:, 0:n])
                rng = slice(F0 + off, F0 + off + n)
                nc.vector.tensor_add(XP[:, rng], XP[:, rng], dxm[:, 0:n])

            if s < steps - 1:     # halo refresh
                for g in range(NB - 1):
                    la, lb = _lane(g), _lane(g + 1)
                    nc.sync.dma_start(XP[lb:lb + 16, 0:WP],
                                      XP[la:la + 16, BH * WP:(BH + 1) * WP])
                for g in range(1, NB):
                    la, lb = _lane(g), _lane(g - 1)
                    nc.sync.dma_start(XP[lb:lb + 16, (BH + 1) * WP:F],
                                      XP[la:la + 16, WP:2 * WP])

        nc.sync.dma_start(out_d.ap(), XP[:, :])
    nc.compile()
    return nc


# mm1 band order: hs[2p]=band 2p, hs[2p+1]=band 2p+1 -> mm2 round1 needs
# bands 0..3 (lane offset +0) and round2 bands 4..7 (+16).
# With lane(g)=32*(g%4)+16*(g//4): round1 strip q <- band q, round2 <- band q+4.
# hs list index for band g: pairs are (0,1),(2,3),(4,5),(6,7) -> hs[g] = band g.
# (hs[q] and hs[4+q] above are exactly bands q and 4+q.)


def _install_ntff_hook():
    """Best-effort: register the axon NTFF profile hook so trace=True works."""
    import types
    try:
        import antenv.axon_hooks  # noqa: F401
        return True
    except ImportError:
        pass
    try:
        import antenv
        from trn_agent_boot.trn_boot import _ntff_profile_via_ctypes
        hook = _ntff_profile_via_ctypes("/opt/axon/libaxon_pjrt.so")
        if hook is None:
            return False
        mod = types.ModuleType("antenv.axon_hooks")
        state = {"hook": hook}
        mod.get_axon_ntff_profile_hook = lambda: state["hook"]
        mod.set_axon_ntff_profile_hook = lambda h: state.update(hook=h)
        sys.modules["antenv.axon_hooks"] = mod
        antenv.axon_hooks = mod
        return True
    except Exception:
        return False


def kernel(**inputs):
    x = np.asarray(inputs["x"], dtype=np.float32)
    W0 = np.asarray(inputs["W0"], dtype=np.float32)
    b0 = np.asarray(inputs["b0"], dtype=np.float32)
    W1 = np.asarray(inputs["W1"], dtype=np.float32)
    steps = int(np.asarray(inputs["steps"]))

    if steps <= 0:
        return x.copy()

    from concourse.bass_utils import run_bass_kernel_spmd

    masks = _masks(steps)
    wt = _tap_weights(W0)
    w2 = _mm2_weights(W1)
    b0t = np.ascontiguousarray(b0.reshape(128, 1).astype(np.float32))

    nc = _build_program(steps)
    in_maps = []
    for b in range(NCORES):
        mask_b = np.stack([_build_mask(masks[s, b]) for s in range(steps)])
        in_maps.append({
            "xp": _build_xp(x[b]),
            "mask": np.ascontiguousarray(mask_b),
            "wt": wt, "w2": w2, "b0": b0t,
        })
    trace = bool(int(os.environ.get("CAK_TRACE", "0")))
    if trace:
        trace = _install_ntff_hook()
    try:
        res = run_bass_kernel_spmd(nc, in_maps, core_ids=list(range(NCORES)),
                                   trace=trace)
    except Exception:
        if not trace:
            raise
        res = run_bass_kernel_spmd(nc, in_maps, core_ids=list(range(NCORES)),
                                   trace=False)
    out = np.stack([_unbuild_xp(res.results[b]["out"]) for b in range(NCORES)])
    kernel.last_exec_time_ns = res.exec_time_ns
    kernel.last_results = res
    return out


# revision 11
# speedup vs baseline: 1.0305x; 1.0305x over previous
"""Trainium2 Bass kernel for the neural-CA model (nn_CAModel_optimizedTraining).

Strategy (per core, data-parallel over batch B=8 across 8 cores):
  - Image [256,256,16] stored in SBUF as 8 horizontal bands (32 rows each,
    plus 1-row halos and 1-col zero pads) packed onto 128 partitions:
    lane(g, c) = 32*(g%4) + 16*(g//4) + c.
  - Sobel convs are separable. DVE computes the two w-direction passes
    S' = x(w-1)+x(w+1) and T = x(w+1)-x(w-1); the h-direction combination
    and the channel-concat matmul are folded into 3 "tap" matmuls
    (dh in {-1,0,+1}) with host-precomputed [48,128] weights over a
    DMA-stacked [x; S'; T] buffer. Row-tiled 2x on the PE (two bands at
    tile_position (0,0) / (64,0)).
  - Hidden relu+bias fused into the PSUM->SBUF pass (ScalarE/VectorE).
  - Second matmul W1@h col-tiled 4x with M=32 zero-padded weights so the
    per-band dx lands densely in one [128,n] PSUM tile (rows 0..2 of W1
    zeroed so fixed channels never change).
  - Stochastic fire mask is input-independent (jax threefry on host),
    uploaded pre-broadcast in the band layout; pad columns get mask 0 so
    padding stays exactly zero.
"""

import os
import sys

import numpy as np

H, W, C, HID = 256, 256, 16, 128
NB = 8            # bands per core
BH = H // NB      # 32 rows per band
WP = W + 2        # padded row width
RPB = BH + 2      # rows per band incl halos
F = RPB * WP      # flat band length (8772)
FI = BH * WP      # interior span (8256)
F0 = WP           # interior start offset
NCORES = 8
FIRE_RATE = 0.5
CHUNK = 512


def _lane(g):
    return 32 * (g % 4) + 16 * (g // 4)


def _chunks():
    out = []
    off = 0
    while off < FI:
        n = min(CHUNK, FI - off)
        out.append((off, n))
        off += n
    return out


# ---------------------------------------------------------------- host prep

def _masks(steps):
    """Reference-exact fire masks: [steps, B, H, W] float32 in {0,1}."""
    import jax
    with jax.default_device(jax.devices("cpu")[0]):
        key = jax.random.key(42)
        ms = []
        for step in range(steps):
            mk = jax.random.fold_in(key, step)
            u = jax.random.uniform(mk, (NCORES, H, W, 1))
            ms.append(np.asarray(u[..., 0] > FIRE_RATE, dtype=np.float32))
    return np.stack(ms) if steps else np.zeros((0, NCORES, H, W), np.float32)


def _build_xp(xb):
    """[H,W,C] -> [128, F] padded band layout."""
    xp = np.zeros((128, F), dtype=np.float32)
    band = np.zeros((NB, RPB, WP, C), dtype=np.float32)
    for g in range(NB):
        band[g, 1:BH + 1, 1:W + 1, :] = xb[32 * g:32 * g + BH]
        if g > 0:
            band[g, 0, 1:W + 1, :] = xb[32 * g - 1]
        if g < NB - 1:
            band[g, BH + 1, 1:W + 1, :] = xb[32 * g + BH]
    flat = band.transpose(0, 3, 1, 2).reshape(NB, C, F)
    for g in range(NB):
        xp[_lane(g):_lane(g) + C, :] = flat[g]
    return xp


def _unbuild_xp(xp):
    """[128, F] -> [H,W,C]."""
    out = np.empty((H, W, C), dtype=np.float32)
    for g in range(NB):
        fl = xp[_lane(g):_lane(g) + C, :].reshape(C, RPB, WP)
        out[32 * g:32 * g + BH] = fl[:, 1:BH + 1, 1:W + 1].transpose(1, 2, 0)
    return out


def _build_mask(mb):
    """[H,W] {0,1} -> [128, FI] band layout (pad cols -> 0), bf16."""
    import ml_dtypes
    mx = np.zeros((128, FI), dtype=np.float32)
    for g in range(NB):
        mrows = np.zeros((BH, WP), dtype=np.float32)
        mrows[:, 1:W + 1] = mb[32 * g:32 * g + BH]
        mx[_lane(g):_lane(g) + C, :] = mrows.reshape(1, FI)
    return mx.astype(ml_dtypes.bfloat16)


def _tap_weights(W0):
    """[128, 3*128] bf16 tap weights (rows 0:48 strip A, 64:112 strip B)."""
    import ml_dtypes
    W0a = W0[:, 0:C].astype(np.float64)
    W0b = W0[:, C:2 * C].astype(np.float64)
    W0c = W0[:, 2 * C:3 * C].astype(np.float64)
    z = np.zeros((C, HID), np.float64)
    taps = [
        np.vstack([(-2 / 8) * W0b.T, (-1 / 8) * W0b.T, (1 / 8) * W0c.T]),   # dh=-1
        np.vstack([W0a.T,            z,                (2 / 8) * W0c.T]),   # dh= 0
        np.vstack([(2 / 8) * W0b.T,  (1 / 8) * W0b.T,  (1 / 8) * W0c.T]),   # dh=+1
    ]
    wt = np.zeros((128, 3 * 128), dtype=np.float32)
    for i, tp in enumerate(taps):
        wt[0:48, 128 * i:128 * i + 128] = tp
        wt[64:112, 128 * i:128 * i + 128] = tp
    return wt.astype(ml_dtypes.bfloat16)


def _mm2_weights(W1):
    """[128, 64] bf16: [:,0:32] round-1 [W1z^T | 0], [:,32:64] round-2."""
    import ml_dtypes
    W1z = W1.astype(np.float32).copy()
    W1z[0:3, :] = 0.0
    w2 = np.zeros((128, 64), dtype=np.float32)
    w2[:, 0:16] = W1z.T
    w2[:, 48:64] = W1z.T
    return w2.astype(ml_dtypes.bfloat16)


# ---------------------------------------------------------------- bass build

def _build_program(steps, h_on_dve_mod=3):
    """Returns (nc, names). h_on_dve_mod: every k-th h-pass op goes to DVE."""
    from contextlib import ExitStack
    import concourse.bass as bass
    import concourse.tile as tile
    from concourse import bacc, mybir

    nc = bacc.Bacc("TRN2", target_bir_lowering=False)
    dt = mybir.dt
    xp_in = nc.dram_tensor("xp", [128, F], dt.float32, kind="ExternalInput")
    mask_in = nc.dram_tensor("mask", [max(steps, 1), 128, FI], dt.bfloat16,
                             kind="ExternalInput")
    wt_in = nc.dram_tensor("wt", [128, 3 * 128], dt.bfloat16, kind="ExternalInput")
    w2_in = nc.dram_tensor("w2", [128, 64], dt.bfloat16, kind="ExternalInput")
    b0_in = nc.dram_tensor("b0", [128, 1], dt.float32, kind="ExternalInput")
    out_d = nc.dram_tensor("out", [128, F], dt.float32, kind="ExternalOutput")

    with tile.TileContext(nc) as tc, ExitStack() as ctx:
        const = ctx.enter_context(tc.tile_pool(name="const", bufs=1))
        state = ctx.enter_context(tc.tile_pool(name="state", bufs=1))
        hpool = ctx.enter_context(tc.tile_pool(name="hpool", bufs=16))
        mpool = ctx.enter_context(tc.tile_pool(name="mpool", bufs=4))
        dpool = ctx.enter_context(tc.tile_pool(name="dpool", bufs=4))
        pshp = ctx.enter_context(tc.tile_pool(name="pshp", bufs=6, space="PSUM"))
        psdx = ctx.enter_context(tc.tile_pool(name="psdx", bufs=2, space="PSUM"))

        wtt = const.tile([128, 3 * 128], dt.bfloat16)
        w2t = const.tile([128, 64], dt.bfloat16)
        b0t = const.tile([128, 1], dt.float32)
        nc.sync.dma_start(wtt[:, :], wt_in.ap())
        nc.sync.dma_start(w2t[:, :], w2_in.ap())
        nc.sync.dma_start(b0t[:, :], b0_in.ap())

        XP = state.tile([128, F], dt.float32)
        XB = state.tile([128, F], dt.bfloat16)     # bf16 mirror of XP
        S = state.tile([128, F], dt.bfloat16)
        T = state.tile([128, F], dt.bfloat16)
        YSTs = [state.tile([128, F], dt.bfloat16, name=f"yst{p}") for p in range(4)]
        nc.sync.dma_start(XP[:, :], xp_in.ap())
        for t_ in (S, T):
            nc.vector.memset(t_[:, 0:1], 0.0)
            nc.vector.memset(t_[:, F - 1:F], 0.0)

        Relu = mybir.ActivationFunctionType.Relu
        Copy = mybir.ActivationFunctionType.Copy
        add_op, max_op = mybir.AluOpType.add, mybir.AluOpType.max
        nc.scalar.activation(XB[:, :], XP[:, :], Copy)

        hctr = 0
        for s in range(steps):
            # w-direction conv passes
            nc.vector.tensor_add(S[:, 1:F - 1], XP[:, 0:F - 2], XP[:, 2:F])
            nc.vector.tensor_sub(T[:, 1:F - 1], XP[:, 2:F], XP[:, 0:F - 2])

            # stack [x; S'; T] per band-pair (strip A rows 0:48, B rows 64:112)
            for p in range(4):
                gA, gB = 2 * p, 2 * p + 1
                for base, g in ((0, gA), (64, gB)):
                    ln = _lane(g)
                    xq = nc.sync if base == 0 else nc.scalar
                    xq.dma_start(YSTs[p][base:base + 16, :], XB[ln:ln + 16, :])
                    nc.sync.dma_start(YSTs[p][base + 16:base + 32, :],
                                      S[ln:ln + 16, :])
                    nc.scalar.dma_start(YSTs[p][base + 32:base + 48, :],
                                        T[ln:ln + 16, :])

            for (off, n) in _chunks():
                mt = mpool.tile([128, CHUNK], dt.bfloat16, tag="mt")
                nc.gpsimd.dma_start(mt[:, 0:n], mask_in.ap()[s, :, off:off + n])

                hs = []
                for p in range(4):
                    pa = pshp.tile([128, CHUNK], dt.float32, tag="ph")
                    pb = pshp.tile([128, CHUNK], dt.float32, tag="ph")
                    for i in range(3):
                        woff = off + 258 * i
                        nc.tensor.matmul(pa[:, 0:n],
                                         wtt[0:48, 128 * i:128 * i + 128],
                                         YSTs[p][0:48, woff:woff + n],
                                         start=(i == 0), stop=(i == 2),
                                         tile_position=(0, 0))
                        nc.tensor.matmul(pb[:, 0:n],
                                         wtt[64:112, 128 * i:128 * i + 128],
                                         YSTs[p][64:112, woff:woff + n],
                                         start=(i == 0), stop=(i == 2),
                                         tile_position=(64, 0))
                    for ps in (pa, pb):
                        ht = hpool.tile([128, CHUNK], dt.bfloat16, tag="h")
                        if h_on_dve_mod and hctr % h_on_dve_mod == 0:
                            nc.vector.tensor_scalar(ht[:, 0:n], ps[:, 0:n],
                                                    b0t[:, 0:1], 0.0, add_op, max_op)
                        else:
                            nc.scalar.activation(ht[:, 0:n], ps[:, 0:n], Relu,
                                                 bias=b0t[:, 0:1])
                        hctr += 1
                        hs.append(ht)

                pdx = psdx.tile([128, CHUNK], dt.float32, tag="pdx")
                for q in range(4):
                    nc.tensor.matmul(pdx[32 * q:32 * q + 32, 0:n], w2t[:, 0:32],
                                     hs[q][:, 0:n], start=True, stop=False,
                                     tile_position=(0, 32 * q),
                                     skip_group_check=True)
                for q in range(4):
                    nc.tensor.matmul(pdx[32 * q:32 * q + 32, 0:n], w2t[:, 32:64],
                                     hs[4 + q][:, 0:n], start=False, stop=True,
                                     tile_position=(0, 32 * q),
                                     skip_group_check=True)

                dxm = dpool.tile([128, CHUNK], dt.float32, tag="dxm")
                nc.vector.tensor_mul(dxm[:, 0:n], pdx[:, 0:n], mt[:, 0:n])
                rng = slice(F0 + off, F0 + off + n)
                nc.gpsimd.tensor_add(XP[:, rng], XP[:, rng], dxm[:, 0:n])
                if s < steps - 1:
                    nc.scalar.activation(XB[:, rng], XP[:, rng], Copy)

            if s < steps - 1:     # halo refresh (XP then XB halo rows)
                for g in range(NB - 1):
                    la, lb = _lane(g), _lane(g + 1)
                    nc.sync.dma_start(XP[lb:lb + 16, 0:WP],
                                      XP[la:la + 16, BH * WP:(BH + 1) * WP])
                for g in range(1, NB):
                    la, lb = _lane(g), _lane(g - 1)
                    nc.sync.dma_start(XP[lb:lb + 16, (BH + 1) * WP:F],
                                      XP[la:la + 16, WP:2 * WP])
                nc.scalar.activation(XB[:, 0:WP], XP[:, 0:WP], Copy)
                nc.scalar.activation(XB[:, (BH + 1) * WP:F], XP[:, (BH + 1) * WP:F],
                                     Copy)

        nc.sync.dma_start(out_d.ap(), XP[:, :])
    nc.compile()
    return nc


# mm1 band order: hs[2p]=band 2p, hs[2p+1]=band 2p+1 -> mm2 round1 needs
# bands 0..3 (lane offset +0) and round2 bands 4..7 (+16).
# With lane(g)=32*(g%4)+16*(g//4): round1 strip q <- band q, round2 <- band q+4.
# hs list index for band g: pairs are (0,1),(2,3),(4,5),(6,7) -> hs[g] = band g.
# (hs[q] and hs[4+q] above are exactly bands q and 4+q.)


def _install_ntff_hook():
    """Best-effort: register the axon NTFF profile hook so trace=True works."""
    import types
    try:
        import antenv.axon_hooks  # noqa: F401
        return True
    except ImportError:
        pass
    try:
        import antenv
        from trn_agent_boot.trn_boot import _ntff_profile_via_ctypes
        hook = _ntff_profile_via_ctypes("/opt/axon/libaxon_pjrt.so")
        if hook is None:
            return False
        mod = types.ModuleType("antenv.axon_hooks")
        state = {"hook": hook}
        mod.get_axon_ntff_profile_hook = lambda: state["hook"]
        mod.set_axon_ntff_profile_hook = lambda h: state.update(hook=h)
        sys.modules["antenv.axon_hooks"] = mod
        antenv.axon_hooks = mod
        return True
    except Exception:
        return False


def kernel(**inputs):
    x = np.asarray(inputs["x"], dtype=np.float32)
    W0 = np.asarray(inputs["W0"], dtype=np.float32)
    b0 = np.asarray(inputs["b0"], dtype=np.float32)
    W1 = np.asarray(inputs["W1"], dtype=np.float32)
    steps = int(np.asarray(inputs["steps"]))

    if steps <= 0:
        return x.copy()

    from concourse.bass_utils import run_bass_kernel_spmd

    masks = _masks(steps)
    wt = _tap_weights(W0)
    w2 = _mm2_weights(W1)
    b0t = np.ascontiguousarray(b0.reshape(128, 1).astype(np.float32))

    nc = _build_program(steps)
    in_maps = []
    for b in range(NCORES):
        mask_b = np.stack([_build_mask(masks[s, b]) for s in range(steps)])
        in_maps.append({
            "xp": _build_xp(x[b]),
            "mask": np.ascontiguousarray(mask_b),
            "wt": wt, "w2": w2, "b0": b0t,
        })
    trace = bool(int(os.environ.get("CAK_TRACE", "0")))
    if trace:
        trace = _install_ntff_hook()
    try:
        res = run_bass_kernel_spmd(nc, in_maps, core_ids=list(range(NCORES)),
                                   trace=trace)
    except Exception:
        if not trace:
            raise
        res = run_bass_kernel_spmd(nc, in_maps, core_ids=list(range(NCORES)),
                                   trace=False)
    out = np.stack([_unbuild_xp(res.results[b]["out"]) for b in range(NCORES)])
    kernel.last_exec_time_ns = res.exec_time_ns
    kernel.last_results = res
    return out


# revision 12
# speedup vs baseline: 1.0426x; 1.0117x over previous
"""Trainium2 Bass kernel for the neural-CA model (nn_CAModel_optimizedTraining).

Strategy (per core, data-parallel over batch B=8 across 8 cores):
  - Image [256,256,16] stored in SBUF as 8 horizontal bands (32 rows each,
    plus 1-row halos and 1-col zero pads) packed onto 128 partitions:
    lane(g, c) = 32*(g%4) + 16*(g//4) + c.
  - Sobel convs are separable. DVE computes the two w-direction passes
    S' = x(w-1)+x(w+1) and T = x(w+1)-x(w-1); the h-direction combination
    and the channel-concat matmul are folded into 3 "tap" matmuls
    (dh in {-1,0,+1}) with host-precomputed [48,128] weights over a
    DMA-stacked [x; S'; T] buffer. Row-tiled 2x on the PE (two bands at
    tile_position (0,0) / (64,0)).
  - Hidden relu+bias fused into the PSUM->SBUF pass (ScalarE/VectorE).
  - Second matmul W1@h col-tiled 4x with M=32 zero-padded weights so the
    per-band dx lands densely in one [128,n] PSUM tile (rows 0..2 of W1
    zeroed so fixed channels never change).
  - Stochastic fire mask is input-independent (jax threefry on host),
    uploaded pre-broadcast in the band layout; pad columns get mask 0 so
    padding stays exactly zero.
"""

import os
import sys

import numpy as np

H, W, C, HID = 256, 256, 16, 128
NB = 8            # bands per core
BH = H // NB      # 32 rows per band
WP = W + 2        # padded row width
RPB = BH + 2      # rows per band incl halos
F = RPB * WP      # flat band length (8772)
FI = BH * WP      # interior span (8256)
F0 = WP           # interior start offset
NCORES = 8
FIRE_RATE = 0.5
CHUNK = 512


def _lane(g):
    return 32 * (g % 4) + 16 * (g // 4)


def _chunks():
    out = []
    off = 0
    while off < FI:
        n = min(CHUNK, FI - off)
        out.append((off, n))
        off += n
    return out


# ---------------------------------------------------------------- host prep

def _masks(steps):
    """Reference-exact fire masks: [steps, B, H, W] float32 in {0,1}."""
    import jax
    with jax.default_device(jax.devices("cpu")[0]):
        key = jax.random.key(42)
        ms = []
        for step in range(steps):
            mk = jax.random.fold_in(key, step)
            u = jax.random.uniform(mk, (NCORES, H, W, 1))
            ms.append(np.asarray(u[..., 0] > FIRE_RATE, dtype=np.float32))
    return np.stack(ms) if steps else np.zeros((0, NCORES, H, W), np.float32)


def _build_xp(xb):
    """[H,W,C] -> [128, F] padded band layout."""
    xp = np.zeros((128, F), dtype=np.float32)
    band = np.zeros((NB, RPB, WP, C), dtype=np.float32)
    for g in range(NB):
        band[g, 1:BH + 1, 1:W + 1, :] = xb[32 * g:32 * g + BH]
        if g > 0:
            band[g, 0, 1:W + 1, :] = xb[32 * g - 1]
        if g < NB - 1:
            band[g, BH + 1, 1:W + 1, :] = xb[32 * g + BH]
    flat = band.transpose(0, 3, 1, 2).reshape(NB, C, F)
    for g in range(NB):
        xp[_lane(g):_lane(g) + C, :] = flat[g]
    return xp


def _unbuild_xp(xp):
    """[128, F] -> [H,W,C]."""
    out = np.empty((H, W, C), dtype=np.float32)
    for g in range(NB):
        fl = xp[_lane(g):_lane(g) + C, :].reshape(C, RPB, WP)
        out[32 * g:32 * g + BH] = fl[:, 1:BH + 1, 1:W + 1].transpose(1, 2, 0)
    return out


def _build_mask(mb):
    """[H,W] {0,1} -> [128, FI] band layout (pad cols -> 0), bf16."""
    import ml_dtypes
    mx = np.zeros((128, FI), dtype=np.float32)
    for g in range(NB):
        mrows = np.zeros((BH, WP), dtype=np.float32)
        mrows[:, 1:W + 1] = mb[32 * g:32 * g + BH]
        mx[_lane(g):_lane(g) + C, :] = mrows.reshape(1, FI)
    return mx.astype(ml_dtypes.bfloat16)


def _tap_weights(W0):
    """[128, 3*128] bf16 tap weights (rows 0:48 strip A, 64:112 strip B)."""
    import ml_dtypes
    W0a = W0[:, 0:C].astype(np.float64)
    W0b = W0[:, C:2 * C].astype(np.float64)
    W0c = W0[:, 2 * C:3 * C].astype(np.float64)
    z = np.zeros((C, HID), np.float64)
    taps = [
        np.vstack([(-2 / 8) * W0b.T, (-1 / 8) * W0b.T, (1 / 8) * W0c.T]),   # dh=-1
        np.vstack([W0a.T,            z,                (2 / 8) * W0c.T]),   # dh= 0
        np.vstack([(2 / 8) * W0b.T,  (1 / 8) * W0b.T,  (1 / 8) * W0c.T]),   # dh=+1
    ]
    wt = np.zeros((128, 3 * 128), dtype=np.float32)
    for i, tp in enumerate(taps):
        wt[0:48, 128 * i:128 * i + 128] = tp
        wt[64:112, 128 * i:128 * i + 128] = tp
    return wt.astype(ml_dtypes.bfloat16)


def _mm2_weights(W1):
    """[128, 64] bf16: [:,0:32] round-1 [W1z^T | 0], [:,32:64] round-2."""
    import ml_dtypes
    W1z = W1.astype(np.float32).copy()
    W1z[0:3, :] = 0.0
    w2 = np.zeros((128, 64), dtype=np.float32)
    w2[:, 0:16] = W1z.T
    w2[:, 48:64] = W1z.T
    return w2.astype(ml_dtypes.bfloat16)


# ---------------------------------------------------------------- bass build

def _build_program(steps, h_on_dve_mod=4):
    """Returns (nc, names). h_on_dve_mod: every k-th h-pass op goes to DVE."""
    from contextlib import ExitStack
    import concourse.bass as bass
    import concourse.tile as tile
    from concourse import bacc, mybir

    nc = bacc.Bacc("TRN2", target_bir_lowering=False)
    dt = mybir.dt
    xp_in = nc.dram_tensor("xp", [128, F], dt.float32, kind="ExternalInput")
    mask_in = nc.dram_tensor("mask", [max(steps, 1), 128, FI], dt.bfloat16,
                             kind="ExternalInput")
    wt_in = nc.dram_tensor("wt", [128, 3 * 128], dt.bfloat16, kind="ExternalInput")
    w2_in = nc.dram_tensor("w2", [128, 64], dt.bfloat16, kind="ExternalInput")
    b0_in = nc.dram_tensor("b0", [128, 1], dt.float32, kind="ExternalInput")
    out_d = nc.dram_tensor("out", [128, F], dt.float32, kind="ExternalOutput")

    with tile.TileContext(nc) as tc, ExitStack() as ctx:
        const = ctx.enter_context(tc.tile_pool(name="const", bufs=1))
        state = ctx.enter_context(tc.tile_pool(name="state", bufs=1))
        hpool = ctx.enter_context(tc.tile_pool(name="hpool", bufs=16))
        mpool = ctx.enter_context(tc.tile_pool(name="mpool", bufs=4))
        dpool = ctx.enter_context(tc.tile_pool(name="dpool", bufs=4))
        pshp = ctx.enter_context(tc.tile_pool(name="pshp", bufs=6, space="PSUM"))
        psdx = ctx.enter_context(tc.tile_pool(name="psdx", bufs=2, space="PSUM"))

        wtt = const.tile([128, 3 * 128], dt.bfloat16)
        w2t = const.tile([128, 64], dt.bfloat16)
        b0t = const.tile([128, 1], dt.float32)
        nc.sync.dma_start(wtt[:, :], wt_in.ap())
        nc.sync.dma_start(w2t[:, :], w2_in.ap())
        nc.sync.dma_start(b0t[:, :], b0_in.ap())

        XP = state.tile([128, F], dt.float32)
        XB = state.tile([128, F], dt.bfloat16)     # bf16 mirror of XP
        S = state.tile([128, F], dt.bfloat16)
        T = state.tile([128, F], dt.bfloat16)
        YSTs = [state.tile([128, F], dt.bfloat16, name=f"yst{p}") for p in range(4)]
        nc.sync.dma_start(XP[:, :], xp_in.ap())
        for t_ in (S, T):
            nc.vector.memset(t_[:, 0:1], 0.0)
            nc.vector.memset(t_[:, F - 1:F], 0.0)

        Relu = mybir.ActivationFunctionType.Relu
        Copy = mybir.ActivationFunctionType.Copy
        add_op, max_op = mybir.AluOpType.add, mybir.AluOpType.max
        nc.scalar.activation(XB[:, :], XP[:, :], Copy)

        hctr = 0
        for s in range(steps):
            # w-direction conv passes
            nc.vector.tensor_add(S[:, 1:F - 1], XP[:, 0:F - 2], XP[:, 2:F])
            nc.gpsimd.tensor_sub(T[:, 1:F - 1], XP[:, 2:F], XP[:, 0:F - 2])

            # stack [x; S'; T] per band-pair (strip A rows 0:48, B rows 64:112)
            for p in range(4):
                gA, gB = 2 * p, 2 * p + 1
                for base, g in ((0, gA), (64, gB)):
                    ln = _lane(g)
                    xq = nc.sync if base == 0 else nc.scalar
                    xq.dma_start(YSTs[p][base:base + 16, :], XB[ln:ln + 16, :])
                    nc.sync.dma_start(YSTs[p][base + 16:base + 32, :],
                                      S[ln:ln + 16, :])
                    nc.scalar.dma_start(YSTs[p][base + 32:base + 48, :],
                                        T[ln:ln + 16, :])

            for (off, n) in _chunks():
                mt = mpool.tile([128, CHUNK], dt.bfloat16, tag="mt")
                nc.sync.dma_start(mt[:, 0:n], mask_in.ap()[s, :, off:off + n])

                hs = []
                for p in range(4):
                    pa = pshp.tile([128, CHUNK], dt.float32, tag="ph")
                    pb = pshp.tile([128, CHUNK], dt.float32, tag="ph")
                    for i in range(3):
                        woff = off + 258 * i
                        nc.tensor.matmul(pa[:, 0:n],
                                         wtt[0:48, 128 * i:128 * i + 128],
                                         YSTs[p][0:48, woff:woff + n],
                                         start=(i == 0), stop=(i == 2),
                                         tile_position=(0, 0))
                        nc.tensor.matmul(pb[:, 0:n],
                                         wtt[64:112, 128 * i:128 * i + 128],
                                         YSTs[p][64:112, woff:woff + n],
                                         start=(i == 0), stop=(i == 2),
                                         tile_position=(64, 0))
                    for ps in (pa, pb):
                        ht = hpool.tile([128, CHUNK], dt.bfloat16, tag="h")
                        if h_on_dve_mod and hctr % h_on_dve_mod == 0:
                            nc.vector.tensor_scalar(ht[:, 0:n], ps[:, 0:n],
                                                    b0t[:, 0:1], 0.0, add_op, max_op)
                        else:
                            nc.scalar.activation(ht[:, 0:n], ps[:, 0:n], Relu,
                                                 bias=b0t[:, 0:1])
                        hctr += 1
                        hs.append(ht)

                pdx = psdx.tile([128, CHUNK], dt.float32, tag="pdx")
                for q in range(4):
                    nc.tensor.matmul(pdx[32 * q:32 * q + 32, 0:n], w2t[:, 0:32],
                                     hs[q][:, 0:n], start=True, stop=False,
                                     tile_position=(0, 32 * q),
                                     skip_group_check=True)
                for q in range(4):
                    nc.tensor.matmul(pdx[32 * q:32 * q + 32, 0:n], w2t[:, 32:64],
                                     hs[4 + q][:, 0:n], start=False, stop=True,
                                     tile_position=(0, 32 * q),
                                     skip_group_check=True)

                dxm = dpool.tile([128, CHUNK], dt.float32, tag="dxm")
                nc.vector.tensor_mul(dxm[:, 0:n], pdx[:, 0:n], mt[:, 0:n])
                rng = slice(F0 + off, F0 + off + n)
                nc.vector.tensor_add(XP[:, rng], XP[:, rng], dxm[:, 0:n])
                if s < steps - 1:
                    nc.scalar.activation(XB[:, rng], XP[:, rng], Copy)

            if s < steps - 1:     # halo refresh (XP then XB halo rows)
                for g in range(NB - 1):
                    la, lb = _lane(g), _lane(g + 1)
                    nc.sync.dma_start(XP[lb:lb + 16, 0:WP],
                                      XP[la:la + 16, BH * WP:(BH + 1) * WP])
                for g in range(1, NB):
                    la, lb = _lane(g), _lane(g - 1)
                    nc.sync.dma_start(XP[lb:lb + 16, (BH + 1) * WP:F],
                                      XP[la:la + 16, WP:2 * WP])
                nc.scalar.activation(XB[:, 0:WP], XP[:, 0:WP], Copy)
                nc.scalar.activation(XB[:, (BH + 1) * WP:F], XP[:, (BH + 1) * WP:F],
                                     Copy)

        nc.sync.dma_start(out_d.ap(), XP[:, :])
    nc.compile()
    return nc


# mm1 band order: hs[2p]=band 2p, hs[2p+1]=band 2p+1 -> mm2 round1 needs
# bands 0..3 (lane offset +0) and round2 bands 4..7 (+16).
# With lane(g)=32*(g%4)+16*(g//4): round1 strip q <- band q, round2 <- band q+4.
# hs list index for band g: pairs are (0,1),(2,3),(4,5),(6,7) -> hs[g] = band g.
# (hs[q] and hs[4+q] above are exactly bands q and 4+q.)


def _install_ntff_hook():
    """Best-effort: register the axon NTFF profile hook so trace=True works."""
    import types
    try:
        import antenv.axon_hooks  # noqa: F401
        return True
    except ImportError:
        pass
    try:
        import antenv
        from trn_agent_boot.trn_boot import _ntff_profile_via_ctypes
        hook = _ntff_profile_via_ctypes("/opt/axon/libaxon_pjrt.so")
        if hook is None:
            return False
        mod = types.ModuleType("antenv.axon_hooks")
        state = {"hook": hook}
        mod.get_axon_ntff_profile_hook = lambda: state["hook"]
        mod.set_axon_ntff_profile_hook = lambda h: state.update(hook=h)
        sys.modules["antenv.axon_hooks"] = mod
        antenv.axon_hooks = mod
        return True
    except Exception:
        return False


def kernel(**inputs):
    x = np.asarray(inputs["x"], dtype=np.float32)
    W0 = np.asarray(inputs["W0"], dtype=np.float32)
    b0 = np.asarray(inputs["b0"], dtype=np.float32)
    W1 = np.asarray(inputs["W1"], dtype=np.float32)
    steps = int(np.asarray(inputs["steps"]))

    if steps <= 0:
        return x.copy()

    from concourse.bass_utils import run_bass_kernel_spmd

    masks = _masks(steps)
    wt = _tap_weights(W0)
    w2 = _mm2_weights(W1)
    b0t = np.ascontiguousarray(b0.reshape(128, 1).astype(np.float32))

    nc = _build_program(steps)
    in_maps = []
    for b in range(NCORES):
        mask_b = np.stack([_build_mask(masks[s, b]) for s in range(steps)])
        in_maps.append({
            "xp": _build_xp(x[b]),
            "mask": np.ascontiguousarray(mask_b),
            "wt": wt, "w2": w2, "b0": b0t,
        })
    trace = bool(int(os.environ.get("CAK_TRACE", "0")))
    if trace:
        trace = _install_ntff_hook()
    try:
        res = run_bass_kernel_spmd(nc, in_maps, core_ids=list(range(NCORES)),
                                   trace=trace)
    except Exception:
        if not trace:
            raise
        res = run_bass_kernel_spmd(nc, in_maps, core_ids=list(range(NCORES)),
                                   trace=False)
    out = np.stack([_unbuild_xp(res.results[b]["out"]) for b in range(NCORES)])
    kernel.last_exec_time_ns = res.exec_time_ns
    kernel.last_results = res
    return out
